# revision 23
# baseline (speedup 1.0000x reference)
"""Bass/Trainium2 kernel for BasicMOE (soft-router MoE with broadcast-bug collapse).

The reference computes
    w = softmax(x @ Wg + bg)                    [B, E]
    y = einsum('bi,eio->beo', x, We) + be       [B, E, O]
    total = einsum('be,beo->o', w, y)           [O]
    out = broadcast(total, [B, O])
which algebraically collapses to
    z = w.T @ x                                 [E, IN]
    s = w.sum(axis=0)                           [E]
    total = einsum('ei,eio->o', z, We) + s @ be [O]
so the kernel never materializes [B, E, O].  The cost is a single streaming
pass over We (1 GiB fp32), expert-sharded across 8 cores (128 MiB/core).

Sharding: We/be sharded on the expert axis (2 experts/core).  x, xt (=x.T),
Wg, bg replicated.  Wg/bg columns are permuted per-core so that the core's
local experts sit at columns 0..1 -- the SPMD program is identical on all
cores.  Each core emits its partial total [O]; the host sums the 8 partials
(the all-reduce) and broadcasts to [B, O].
"""

import numpy as np

import concourse.bass as bass
import concourse.mybir as mybir
import concourse.tile as tile
from concourse.bass_utils import run_bass_kernel_spmd
from concourse.masks import make_identity

B, IN, OUT, E = 1024, 4096, 4096, 16
NCORES = 8
EPC = E // NCORES          # experts per core = 2
KPC = EPC * IN             # contraction rows per core = 8192
NKT = KPC // 128           # we k-tiles per core = 64
NIT = IN // 128            # i-tiles = 32
NBT = B // 128             # b-tiles = 8
NOC = OUT // 512           # output chunks = 8
FP = mybir.dt.float32

# DMA batching for the We stream: K-tiles per slab DMA.
SLAB_KT = 2                # 4 MiB per dma_start
SLAB_BUFS = 3
X_BUFS = 2
XT_BUFS = 3


def _moe_device_program(nc, x_d, xt_d, wg_d, bg_d, we_d, be_d, out_d):
    with tile.TileContext(nc) as tc:
        with (
            tc.tile_pool(name="singles", bufs=1) as singles,
            tc.tile_pool(name="xt_pool", bufs=XT_BUFS) as xt_pool,
            tc.tile_pool(name="x_pool", bufs=X_BUFS) as x_pool,
            tc.tile_pool(name="w_pool", bufs=2) as w_pool,
            tc.tile_pool(name="small", bufs=4) as small,
            tc.tile_pool(name="slab_pool", bufs=SLAB_BUFS) as slab_pool,
        ):
            # ---- constants / small resident tensors ----
            wg_sbuf = singles.tile([128, NIT, E], FP)          # [128, 32, 16]
            nc.gpsimd.dma_start(
                out=wg_sbuf, in_=wg_d.rearrange("(t p) e -> p t e", p=128)
            )
            bg_sbuf = singles.tile([E, 1], FP)
            nc.gpsimd.dma_start(out=bg_sbuf, in_=bg_d)
            be_sbuf = singles.tile([EPC, OUT], FP)
            nc.gpsimd.dma_start(out=be_sbuf, in_=be_d)
            ident = singles.tile([E, E], FP)
            make_identity(nc, ident)
            ones128 = singles.tile([128, 1], FP)
            nc.vector.memset(ones128, 1.0)

            logitsT_s = singles.tile([E, B], FP)               # [16, 1024]
            z_sbuf = singles.tile([E, IN], FP)                 # [16, 4096]
            zT_sbuf = singles.tile([128, NIT, E], FP)          # [128, 32, 16]
            s_sbuf = singles.tile([E, 1], FP)
            out_sbuf = singles.tile([1, OUT], FP)              # [1, 4096]

            with (
                tc.tile_pool(name="psA", bufs=1, space="PSUM") as psA_pool,
                tc.tile_pool(name="ps_tr", bufs=2, space="PSUM") as tr_pool,
                tc.tile_pool(name="ps_s", bufs=1, space="PSUM") as s_pool,
                tc.tile_pool(name="ps_z", bufs=2, space="PSUM") as z_pool,
                tc.tile_pool(name="ps_warm", bufs=1, space="PSUM") as warm_pool,
            ):
                # Scratch PSUM target for "wait absorber" matmuls.  The PE's
                # LDWEIGHTS slot encodes a single semaphore wait, so any
                # matmul whose operands complete on two different semaphores
                # fails walrus codegen.  Each absorber below reads exactly one
                # not-yet-observed producer so the real matmuls that follow
                # need at most one wait.
                warm = warm_pool.tile([E, E], FP)

                def absorb(src_col):
                    return nc.tensor.matmul(
                        warm[0:1, 0:1], src_col, src_col,
                        start=True, stop=True, skip_group_check=True,
                    )

                absorb(wg_sbuf[:, 0, 0:1])          # wg DMA
                nc.tensor.transpose(warm, ident, ident)  # ident (gpsimd)

                # ---- Phase A: logitsT = Wg.T @ x.T, accumulated over i-tiles
                lgA = psA_pool.tile([E, B], FP)                # [16, 1024] - 2 banks
                for it in range(NIT):
                    xt_tile = xt_pool.tile([128, B], FP)
                    nc.gpsimd.dma_start(
                        out=xt_tile, in_=xt_d[it * 128:(it + 1) * 128, :]
                    )
                    for c in range(B // 512):
                        nc.tensor.matmul(
                            lgA[:, c * 512:(c + 1) * 512],
                            wg_sbuf[:, it, :],
                            xt_tile[:, c * 512:(c + 1) * 512],
                            start=(it == 0),
                            stop=(it == NIT - 1),
                        )
                # bias add (per-partition scalar) + copy PSUM -> SBUF
                nc.vector.tensor_scalar_add(logitsT_s, lgA, bg_sbuf)

                # ---- Phase B: per b-tile transpose + softmax + z/s partials
                s_ps = s_pool.tile([E, 1], FP)
                for bt in range(NBT):
                    x_tile = x_pool.tile([128, IN], FP)
                    nc.gpsimd.dma_start(
                        out=x_tile, in_=x_d[bt * 128:(bt + 1) * 128, :]
                    )
                    absorb(x_tile[:, 0:1])          # x DMA
                    lg_ps = tr_pool.tile([128, E], FP, tag="tr")
                    nc.tensor.transpose(
                        lg_ps, logitsT_s[:, bt * 128:(bt + 1) * 128], ident
                    )
                    mx = small.tile([128, 1], FP)
                    nc.vector.reduce_max(mx, lg_ps, axis=mybir.AxisListType.X)
                    negmx = small.tile([128, 1], FP)
                    nc.vector.tensor_scalar_mul(negmx, mx, -1.0)
                    wexp = small.tile([128, E], FP)
                    ssum = small.tile([128, 1], FP)
                    nc.scalar.activation(
                        wexp, lg_ps, mybir.ActivationFunctionType.Exp,
                        bias=negmx, accum_out=ssum,
                    )
                    rc = small.tile([128, 1], FP)
                    nc.vector.reciprocal(rc, ssum)
                    w_tile = w_pool.tile([128, E], FP)
                    nc.vector.tensor_scalar_mul(w_tile, wexp, rc)

                    # s partial: s += w_tile.T @ ones
                    nc.tensor.matmul(
                        s_ps, w_tile, ones128,
                        start=(bt == 0), stop=(bt == NBT - 1),
                    )
                    # z partial: z[e, i] += w.T @ x   (accumulated in SBUF)
                    for c in range(IN // 512):
                        z_ps = z_pool.tile([E, 512], FP, tag="zc")
                        nc.tensor.matmul(
                            z_ps, w_tile, x_tile[:, c * 512:(c + 1) * 512]
                        )
                        if bt == 0:
                            nc.vector.tensor_copy(
                                z_sbuf[:, c * 512:(c + 1) * 512], z_ps
                            )
                        else:
                            nc.vector.tensor_add(
                                z_sbuf[:, c * 512:(c + 1) * 512],
                                z_sbuf[:, c * 512:(c + 1) * 512],
                                z_ps,
                            )
                # ---- Phase C: zT tiles via PE transpose
                absorb(z_sbuf[:, 0:1])              # DVE z accumulation
                for it in range(NIT):
                    zt_ps = tr_pool.tile([128, E], FP, tag="tr")
                    nc.tensor.transpose(
                        zt_ps, z_sbuf[:, it * 128:(it + 1) * 128], ident
                    )
                    nc.vector.tensor_copy(zT_sbuf[:, it, :], zt_ps)
                # Keep the s copy as the LAST DVE op before phase D so a
                # single absorber of s_sbuf observes every DVE write above.
                nc.vector.tensor_copy(s_sbuf, s_ps)
                a_be = absorb(be_sbuf[0:1, 0:1])    # be DMA
                a_s = absorb(s_sbuf[0:1, :])        # DVE s copy
                a_zt = absorb(zT_sbuf[:, NIT - 1, 0:1])  # DVE last zT copy

            # ---- Phase D: total[o] = sum_k z[k] We[k, o]  (+ s @ be)
            with tc.tile_pool(name="ps_tot", bufs=NOC, space="PSUM") as tot_pool:
                tots = [
                    tot_pool.tile([1, 512], FP, name=f"tot{ot}", tag="tot")
                    for ot in range(NOC)
                ]

                from concourse.tile_rust import add_dep_helper
                for ot in range(NOC):
                    bm = nc.tensor.matmul(
                        tots[ot], s_sbuf[0:EPC, :],
                        be_sbuf[:, ot * 512:(ot + 1) * 512],
                        start=True, stop=False,
                    )
                    if ot == 0:
                        # Keep the wait-absorbers ahead of the first phase-D
                        # matmul so it needs only the PSUM bank-release wait.
                        add_dep_helper(bm.ins, a_be.ins, False)
                        add_dep_helper(bm.ins, a_s.ins, False)
                        add_dep_helper(bm.ins, a_zt.ins, False)
                we_r = we_d.rearrange("(n a p) o -> n p a o", a=SLAB_KT, p=128)
                for n in range(NKT // SLAB_KT):
                    slab = slab_pool.tile([128, SLAB_KT, OUT], FP)
                    nc.gpsimd.dma_start(out=slab, in_=we_r[n])
                    for a in range(SLAB_KT):
                        k = n * SLAB_KT + a
                        e, it = divmod(k, NIT)
                        zcol = zT_sbuf[:, it, e:e + 1]
                        for ot in range(NOC):
                            nc.tensor.matmul(
                                tots[ot],
                                zcol,
                                slab[:, a, ot * 512:(ot + 1) * 512],
                                start=False,
                                stop=(k == NKT - 1),
                            )
                for ot in range(NOC):
                    nc.vector.tensor_copy(
                        out_sbuf[:, ot * 512:(ot + 1) * 512], tots[ot]
                    )
                nc.gpsimd.dma_start(out=out_d, in_=out_sbuf)


def _split_multi_waits(nc, keep=1):
    """Walrus encodes at most one semaphore wait per TPB instruction struct
    (S3_LW for matmul, PSEUDO_DMA_DIRECT2D for DMA, ...).  Tile's scheduler
    sometimes attaches 2-3 waits to one instruction; hoist the extras onto
    standalone same-engine EventSemaphore waits placed just before it --
    semantically identical (the engine sequencer blocks on them in order).
    """
    n = 0
    for f in nc.m.functions:
        for blk in f.blocks:
            new_insts = []
            for inst in blk.instructions:
                si = getattr(inst, "sync_info", None)
                waits = list(si.on_wait) if si and si.on_wait else []
                if len(waits) > keep:
                    for w in waits[:-keep]:
                        ev = mybir.InstEventSemaphore(
                            name=f"presplit_{n}_{inst.name}", ins=[], outs=[]
                        )
                        n += 1
                        ev.engine = inst.engine
                        ev.sync_info = mybir.SyncInfo(on_wait=[w], on_update=[])
                        ev.bass_nofuse = True
                        new_insts.append(ev)
                    si.on_wait = waits[-keep:]
                new_insts.append(inst)
            blk.instructions = new_insts
    return n


def build_bass(split_waits=True):
    nc = bass.Bass("TRN2", target_bir_lowering=False, num_devices=NCORES)
    x_d = nc.dram_tensor("x", [B, IN], FP, kind="ExternalInput").ap()
    xt_d = nc.dram_tensor("xt", [IN, B], FP, kind="ExternalInput").ap()
    wg_d = nc.dram_tensor("wg", [IN, E], FP, kind="ExternalInput").ap()
    bg_d = nc.dram_tensor("bg", [E, 1], FP, kind="ExternalInput").ap()
    we_d = nc.dram_tensor("we", [KPC, OUT], FP, kind="ExternalInput").ap()
    be_d = nc.dram_tensor("be", [EPC, OUT], FP, kind="ExternalInput").ap()
    out_d = nc.dram_tensor("out", [1, OUT], FP, kind="ExternalOutput").ap()
    _moe_device_program(nc, x_d, xt_d, wg_d, bg_d, we_d, be_d, out_d)
    if split_waits:
        _split_multi_waits(nc)
    return nc


def make_in_maps(x, Wg, bg, We, be):
    x = np.ascontiguousarray(np.asarray(x, dtype=np.float32))
    Wg = np.asarray(Wg, dtype=np.float32)
    bg = np.asarray(bg, dtype=np.float32)
    We = np.asarray(We, dtype=np.float32)
    be = np.asarray(be, dtype=np.float32)
    xt = np.ascontiguousarray(x.T)
    in_maps = []
    for c in range(NCORES):
        loc = list(range(EPC * c, EPC * (c + 1)))
        perm = loc + [e for e in range(E) if e not in loc]
        in_maps.append({
            "x": x,
            "xt": xt,
            "wg": np.ascontiguousarray(Wg[:, perm]),
            "bg": np.ascontiguousarray(bg[perm]).reshape(E, 1),
            "we": np.ascontiguousarray(We[loc[0]:loc[-1] + 1].reshape(KPC, OUT)),
            "be": np.ascontiguousarray(be[loc[0]:loc[-1] + 1]),
        })
    return in_maps


_NC_CACHE = None


def _get_nc():
    global _NC_CACHE
    if _NC_CACHE is None:
        _NC_CACHE = build_bass()
    return _NC_CACHE


def kernel(x, Wg, bg, We, be, **_ignored):
    in_maps = make_in_maps(x, Wg, bg, We, be)
    nc = _get_nc()
    res = run_bass_kernel_spmd(nc, in_maps, core_ids=list(range(NCORES)))
    total = np.zeros(OUT, dtype=np.float32)
    for r in res.results:
        total = total + r["out"].reshape(OUT).astype(np.float32)
    return np.ascontiguousarray(
        np.broadcast_to(total, (B, OUT)).astype(np.float32)
    )
